# revision 54
# baseline (speedup 1.0000x reference)
"""Trainium2 Bass kernel for a DehazeBlock:
    res1 = relu(conv3x3(x, w1) + b1) + x
    res2 = conv3x3(res1, w2) + b2
    out  = deform_conv(res2, p_w, p_b, dw) + x

Sharding: 8 cores = 4 batch x 2 H-halves (32 rows each, data-parallel,
communication-free; each core gets a zero-padded 40-row input slab).

Deformable conv strategy (all-PE, no gathers): since offsets |t| < 1, a
bilinear sample at (base + t) decomposes over a 3-tap stencil with weights
(relu(-t), 1-|t|, relu(t)).  Folding the per-tap 256x256 channel mix first
(y_n = DW_n @ res2 over all padded pixels), the output becomes, per tap, a
banded matmul  out[m,u] += sum_v y_n[v,m] * B_n[v,u]  where B_n has 9
diagonals holding the per-pixel stencil weights.  B_n is built on-chip with
iota + local_scatter from weight planes assembled by shifted-DMA reads of a
skew-stored DRAM staging layout.

v3 restructure (340.5us -> ~227us):
 - y-tile channel mixes run as PE filler overlapping the DRAM staging round
   trip (dedicated SBUF pool disjoint from the staging scratch).  The psy
   accumulation uses five 1-bank 512-col psum chunks (2 pre-opened before
   the offset-conv pool + 3 after, to fit 8 banks in every phase) with
   cb-outer runs of 5 same-lhsT matmuls (amortized LDWEIGHTS) and per-chunk
   casts alternating DVE/ACT, so each slot frees after one fast 512-col
   cast and the PE y-pipeline runs at ~1.8us/vtile instead of ~3us.
 - each vtile's y/B5 built exactly once (global cache, was 25 -> 19 builds).
 - startup DMAs chunked and spread across sync/scalar/gpsimd issue queues
   so the first conv matmul starts at ~13us instead of ~25us.
 - staging skew-store DMAs merged (F/G: 18 -> 6); the sw multiply runs on
   gpsimd so the scatter chain is not queued behind the cast stream; the
   whole staging+build chain is high_priority.
 - pool scopes arranged so no SBUF release emitted after the y-loop gates
   the b5 pools (pwt/offs/prt/pct live in a pool the b5 tiles never reuse;
   conv-only tiles release right after conv2).
 - residuals read the bf16 x slab directly (no f32 x copy: -2.7MB of H2D
   and -21KB SBUF); conv eviction scratch in bf16 for 2x DVE throughput.
 - fdram/gdram zeroed from a host-provided zeros tensor (DRAM->DRAM).
"""

import os
import numpy as np
import ml_dtypes

import bass_rust
import concourse.bass as bass
import concourse.mybir as mybir
import concourse.tile as tile
from concourse import bacc
from concourse.bass_utils import run_bass_kernel_spmd
from concourse.masks import make_identity

bf16 = ml_dtypes.bfloat16
F32 = mybir.dt.float32
BF = mybir.dt.bfloat16
I16 = mybir.dt.int16

P = 128
CB = 2              # channel blocks (256 = 2*128)
W = 66              # padded row width
TS = 40             # x slab rows
R1 = 38             # res1 rows
R2 = 36             # res2 rows (= v rows)
RO = 32             # output rows per core
U = RO * W          # 2112 output pixel space
XL = TS * W + 2     # 2642 padded flat x row-span (+1 lead, +1 tail elem)
R1L = R1 * W + 2    # 2510
VT = 19             # v tiles
VPW = VT * P        # 2432
FW = 2720           # staging row width for F/G planes
MARG = 266
BW = 262            # banded-matrix u-window width per (tap, vtile)
TAPS = 9
N_CORES = 8

CONV1_CHUNKS = [(0, 7), (7, 7), (14, 7), (21, 7), (28, 7), (35, 3)]
CONV2_CHUNKS = [(0, 6), (6, 6), (12, 6), (18, 6), (24, 6), (30, 6)]
OFFS_CHUNKS = [(0, 7), (7, 7), (14, 7), (21, 7), (28, 4)]
UBLOCKS = [(0, 11), (11, 11), (22, 10)]  # (row start, rows)

XSB_SPLIT = 6       # startup DMA chunks per xsb channel block
DWT_SPLIT = 3

_CACHE = {}
LAST_RESULTS = None


def _mk_src(t, dims, off):
    s = t.ap().copy()
    s.ap = bass_rust.VecI64Pair(dims)
    s.offset = off
    return s


def _split_at_banks(lo, hi, base):
    """Split [lo, hi) (psum-tile-relative) at 512-elem bank boundaries."""
    segs = []
    a = lo
    while a < hi:
        b = min(hi, ((a - base) // 512 + 1) * 512 + base)
        segs.append((a, b))
        a = b
    return segs


def _ublock_mms(ob, rb):
    U0, UW = ob * W, rb * W
    mms = []
    for vt in range(VT):
        v0 = vt * P
        for n in range(TAPS):
            nr, ncc = n // 3 - 1, n % 3 - 1
            w0 = v0 - 199 - 66 * nr - ncc
            lo, hi = max(w0, U0), min(w0 + BW, U0 + UW)
            if lo >= hi:
                continue
            mms.append((vt, n, w0, lo, hi))
    return mms


def _build_program():
    nc = bacc.Bacc("TRN2", target_bir_lowering=False, debug=False,
                   num_devices=N_CORES)

    # ---------------- dram I/O ----------------
    xsb_d = nc.dram_tensor("xsb", [CB, P, XL], BF, kind="ExternalInput")
    maskr_d = nc.dram_tensor("maskr", [P, TS], BF, kind="ExternalInput")
    w1t_d = nc.dram_tensor("w1t", [CB, TAPS, P, 256], BF, kind="ExternalInput")
    w2t_d = nc.dram_tensor("w2t", [CB, TAPS, P, 256], BF, kind="ExternalInput")
    pwt_d = nc.dram_tensor("pwt", [CB, TAPS, P, 18], BF, kind="ExternalInput")
    dwt_d = nc.dram_tensor("dwt", [CB, P, TAPS * 256], BF, kind="ExternalInput")
    b1_d = nc.dram_tensor("b1", [CB, P, 1], F32, kind="ExternalInput")
    b2_d = nc.dram_tensor("b2", [CB, P, 1], F32, kind="ExternalInput")
    pb_d = nc.dram_tensor("pb", [18, 1], F32, kind="ExternalInput")
    zeros_d = nc.dram_tensor("zeros", [36, FW], BF, kind="ExternalInput")
    out_d = nc.dram_tensor("out", [CB, P, RO, 64], F32, kind="ExternalOutput")

    fdram = nc.dram_tensor("fdram", [27, FW], BF)
    gdram = nc.dram_tensor("gdram", [36, FW], BF)

    RELU = mybir.ActivationFunctionType.Relu
    ABS = mybir.ActivationFunctionType.Abs
    IDENT = mybir.ActivationFunctionType.Identity
    MUL = mybir.AluOpType.mult
    ADD = mybir.AluOpType.add

    with tile.TileContext(nc) as tc:
        with tc.tile_pool(name="perm", bufs=1) as perm, \
             tc.tile_pool(name="ypool", bufs=1) as ypool, \
             tc.tile_pool(name="chunk", bufs=3) as chunk, \
             tc.tile_pool(name="outp", bufs=2) as outp, \
             tc.tile_pool(name="prtp", bufs=1) as prtp, \
             tc.tile_pool(name="tpsum", bufs=1, space="PSUM") as tpsum:

            # ---------------- permanent residents ----------------
            xsb = [perm.tile([P, XL], BF, name=f"xsb{c}") for c in range(CB)]
            dwt = [perm.tile([P, TAPS * 256], BF, name=f"dwt{c}") for c in range(CB)]
            res2b = [perm.tile([P, VPW], BF, name=f"res2b{c}") for c in range(CB)]
            sw = perm.tile([112, VPW], BF, name="sw")
            ident = perm.tile([P, P], BF, name="ident")
            iotaA = perm.tile([P, 60], I16, name="iotaA")
            iotaB = perm.tile([P, 48], I16, name="iotaB")
            maskr = perm.tile([P, TS], BF, name="maskr")
            b1 = [perm.tile([P, 1], F32, name=f"b1{c}") for c in range(CB)]
            b2 = [perm.tile([P, 1], F32, name=f"b2{c}") for c in range(CB)]
            pb = perm.tile([18, 1], F32, name="pb")

            # startup DMAs: critical path (xsb0 + w1t cb0) first on sync;
            # the rest spread over idle engine queues for parallel issue.
            xb = [0]
            while xb[-1] < XL:
                xb.append(min(XL, xb[-1] + (XL + XSB_SPLIT - 1) // XSB_SPLIT))
            for a, b in zip(xb[:-1], xb[1:]):
                nc.sync.dma_start(
                    out=xsb[0][:, a:b],
                    in_=_mk_src(xsb_d, [[XL, P], [1, b - a]], a))
            nc.scalar.dma_start(out=b1[0][:], in_=b1_d.ap()[0])
            nc.scalar.dma_start(out=b1[1][:], in_=b1_d.ap()[1])
            nc.scalar.dma_start(out=maskr[:], in_=maskr_d.ap())

            nc.gpsimd.dma_start(out=fdram.ap(),
                                in_=_mk_src(zeros_d, [[FW, 27], [1, FW]], 0))
            nc.gpsimd.dma_start(out=gdram.ap(), in_=zeros_d.ap())
            db = [0]
            while db[-1] < TAPS * 256:
                db.append(min(TAPS * 256,
                              db[-1] + (TAPS * 256 + DWT_SPLIT - 1) // DWT_SPLIT))
            for c in range(CB):
                for a, b in zip(db[:-1], db[1:]):
                    nc.scalar.dma_start(
                        out=dwt[c][:, a:b],
                        in_=_mk_src(dwt_d, [[TAPS * 256, P], [1, b - a]],
                                    c * P * TAPS * 256 + a))
            nc.scalar.dma_start(out=pb[:], in_=pb_d.ap())
            nc.scalar.dma_start(out=b2[0][:], in_=b2_d.ap()[0])
            nc.scalar.dma_start(out=b2[1][:], in_=b2_d.ap()[1])

            make_identity(nc, ident[:])
            nc.gpsimd.iota(iotaA[:], pattern=[[BW, 5], [-66, 3], [1, 4]],
                           base=131, channel_multiplier=1)
            nc.gpsimd.iota(iotaB[:], pattern=[[BW, 4], [-66, 3], [1, 4]],
                           base=131, channel_multiplier=1)
            for c in range(CB):
                nc.vector.memset(res2b[c][:, R2 * W:VPW], 0)

            ys = []

            # ============ phases 1-4 (freed before phase 5) ============
            # pwt/offs live in prtp (top-level, never reused by b5) so that
            # no SBUF release emitted after the y-loop gates the b5 pools.
            pwt = [prtp.tile([P, TAPS * 18], BF, name=f"pwt{c}") for c in range(CB)]
            offs = prtp.tile([18, U], BF, name="offs")
            convpA_cm = tc.tile_pool(name="convpA", bufs=1)
            convpA = convpA_cm.__enter__()
            if True:
                w1t = [convpA.tile([P, TAPS * 256], BF, name=f"w1t{c}") for c in range(CB)]
                w2t = [convpA.tile([P, TAPS * 256], BF, name=f"w2t{c}") for c in range(CB)]
                res1b = [convpA.tile([P, R1L], BF, name=f"res1b{c}") for c in range(CB)]

                # w1t on sync (behind xsb0, ahead of xsb1 need); w2t/zeros on
                # gpsimd (their ~1us-each sim cost lands well before the
                # staging reads); scalar stays short so conv1 evictions of
                # mb=0 are not queued behind a DMA-issue glut.
                for t in range(TAPS):
                    nc.sync.dma_start(out=w1t[0][:, t * 256:(t + 1) * 256],
                                      in_=w1t_d.ap()[0, t])
                for a, b in zip(xb[:-1], xb[1:]):
                    nc.sync.dma_start(
                        out=xsb[1][:, a:b],
                        in_=_mk_src(xsb_d, [[XL, P], [1, b - a]], P * XL + a))
                for t in range(TAPS):
                    nc.sync.dma_start(out=w1t[1][:, t * 256:(t + 1) * 256],
                                      in_=w1t_d.ap()[1, t])
                for c in range(CB):
                    for t in range(TAPS):
                        nc.gpsimd.dma_start(out=w2t[c][:, t * 256:(t + 1) * 256],
                                            in_=w2t_d.ap()[c, t])
                    nc.scalar.dma_start(
                        out=pwt[c][:].rearrange("p (t m) -> p t m", m=18),
                        in_=_mk_src(pwt_d, [[18, P], [P * 18, TAPS], [1, 18]],
                                    c * TAPS * P * 18))
                    nc.vector.memset(res1b[c][:, 0:1], 0)
                    nc.vector.memset(res1b[c][:, R1L - 1:R1L], 0)

                # ---- phase 1: conv1 -> res1b ----
                with tc.tile_pool(name="cpsum", bufs=6, space="PSUM") as cpsum:
                    for mb in range(CB):
                        psums = [cpsum.tile([P, 462], F32, tag="c1ps",
                                            name=f"c1ps_{mb}_{i}")
                                 for i in range(len(CONV1_CHUNKS))]
                        for cb in range(CB):
                            for t in range(TAPS):
                                ky, kx = t // 3, t % 3
                                lhsT = w1t[cb][:, t * 256 + mb * P:
                                               t * 256 + mb * P + P]
                                first = (cb == 0 and t == 0)
                                last = (cb == CB - 1 and t == TAPS - 1)
                                for ci, (c0, cr) in enumerate(CONV1_CHUNKS):
                                    o = 1 + (c0 + ky) * W + kx - 1
                                    nc.tensor.matmul(
                                        psums[ci][:, :cr * W], lhsT,
                                        xsb[cb][:, o:o + cr * W],
                                        start=first, stop=last)
                        for ci, (c0, cr) in enumerate(CONV1_CHUNKS):
                            n = cr * W
                            tmp = chunk.tile([P, 462], BF, tag="post")
                            nc.scalar.activation(tmp[:, :n], psums[ci][:, :n],
                                                 RELU, bias=b1[mb][:], scale=1.0)
                            nc.vector.tensor_add(
                                tmp[:, :n], tmp[:, :n],
                                xsb[mb][:, 1 + (c0 + 1) * W: 1 + (c0 + 1) * W + n])
                            mv = maskr[:, c0 + 1:c0 + 1 + cr, None] \
                                .to_broadcast((P, cr, W))
                            ov = res1b[mb][:, 1 + c0 * W: 1 + (c0 + cr) * W]
                            nc.vector.tensor_tensor(
                                ov.rearrange("p (r w) -> p r w", w=W),
                                tmp[:, :n].rearrange("p (r w) -> p r w", w=W),
                                mv, MUL)
                    for c in range(CB):
                        v = res1b[c][:, 1:1 + R1 * W].rearrange(
                            "p (r w) -> p r w", w=W)
                        nc.vector.memset(v[:, :, 0:1], 0)
                        nc.vector.memset(v[:, :, 65:66], 0)

                # ---- phase 2: conv2 -> res2b ----
                with tc.tile_pool(name="c2psum", bufs=6, space="PSUM") as c2psum:
                    for mb in range(CB):
                        psums = [c2psum.tile([P, 396], F32, tag="c2ps",
                                             name=f"c2ps_{mb}_{i}")
                                 for i in range(len(CONV2_CHUNKS))]
                        for cb in range(CB):
                            for t in range(TAPS):
                                ky, kx = t // 3, t % 3
                                lhsT = w2t[cb][:, t * 256 + mb * P:
                                               t * 256 + mb * P + P]
                                first = (cb == 0 and t == 0)
                                last = (cb == CB - 1 and t == TAPS - 1)
                                for ci, (e0, cr) in enumerate(CONV2_CHUNKS):
                                    o = 1 + (e0 + ky) * W + kx - 1
                                    nc.tensor.matmul(
                                        psums[ci][:, :cr * W], lhsT,
                                        res1b[cb][:, o:o + cr * W],
                                        start=first, stop=last)
                        for ci, (e0, cr) in enumerate(CONV2_CHUNKS):
                            n = cr * W
                            tmp = chunk.tile([P, 462], BF, tag="post")
                            nc.scalar.activation(tmp[:, :n], psums[ci][:, :n],
                                                 IDENT, bias=b2[mb][:], scale=1.0)
                            mv = maskr[:, e0 + 2:e0 + 2 + cr, None] \
                                .to_broadcast((P, cr, W))
                            ov = res2b[mb][:, e0 * W:(e0 + cr) * W]
                            nc.vector.tensor_tensor(
                                ov.rearrange("p (r w) -> p r w", w=W),
                                tmp[:, :n].rearrange("p (r w) -> p r w", w=W),
                                mv, MUL)
                    for c in range(CB):
                        v = res2b[c][:, 0:R2 * W].rearrange("p (r w) -> p r w", w=W)
                        nc.vector.memset(v[:, :, 0:1], 0)
                        nc.vector.memset(v[:, :, 65:66], 0)

                # conv-only tiles are dead now: release their SBUF early so
                # the b5 pools can allocate without waiting on phase 4.
                convpA_cm.__exit__(None, None, None)

                # ypsA opens before opsum so its 2 banks land on banks freed
                # by cpsum/c2psum (long released when psy matmuls start).
                # ypsB opens after opsum closes (phase-3 release lag expires
                # before its first use).
                ypsA_cm = tc.tile_pool(name="ypsA", bufs=2, space="PSUM")
                ypsA = ypsA_cm.__enter__()

                # ---- phase 3: offset conv -> offs ----
                with tc.tile_pool(name="opsum", bufs=5, space="PSUM") as opsum:
                    psums = [opsum.tile([18, 462], F32, tag="ops",
                                        name=f"ops_{i}")
                             for i in range(len(OFFS_CHUNKS))]
                    for cb in range(CB):
                        for t in range(TAPS):
                            ky, kx = t // 3, t % 3
                            lhsT = pwt[cb][:, t * 18:(t + 1) * 18]
                            first = (cb == 0 and t == 0)
                            last = (cb == CB - 1 and t == TAPS - 1)
                            for ci, (i0, cr) in enumerate(OFFS_CHUNKS):
                                o = (i0 + 1 + ky) * W + kx - 1
                                nc.tensor.matmul(
                                    psums[ci][:, :cr * W], lhsT,
                                    res2b[cb][:, o:o + cr * W],
                                    start=first, stop=last)
                    for ci, (i0, cr) in enumerate(OFFS_CHUNKS):
                        n = cr * W
                        nc.scalar.activation(offs[:, i0 * W:i0 * W + n],
                                             psums[ci][:, :n], IDENT,
                                             bias=pb[:], scale=1.0)

                ypsB_cm = tc.tile_pool(name="ypsB", bufs=3, space="PSUM")
                ypsB = ypsB_cm.__enter__()

                # ---- phase 4: F/G planes + staging round trip ----
                with tc.tile_pool(name="scratch", bufs=1) as scratch, \
                     tc.high_priority():
                    # off_c must start at partition 0 for engine ops
                    offc = scratch.tile([9, U], BF, tag="s4")
                    nc.sync.dma_start(out=offc[:], in_=offs[9:18, :])
                    fsb = scratch.tile([73, U], BF, tag="s1")
                    gsb = scratch.tile([73, U], BF, tag="s2")
                    tmpa = scratch.tile([9, U], BF, tag="s3")
                    tmpb = scratch.tile([9, U], BF, tag="s5")
                    # F/G plane math: DVE takes the cheap ts forms, ACT the abs
                    nc.vector.tensor_scalar(fsb[0:9, :], offs[0:9, :],
                                            -1.0, 0.0, MUL,
                                            mybir.AluOpType.max)
                    nc.scalar.activation(tmpa[0:9, :], offs[0:9, :], ABS)
                    nc.vector.tensor_scalar(fsb[32:41, :], tmpa[0:9, :],
                                            -1.0, 1.0, MUL, ADD)
                    nc.vector.tensor_scalar(fsb[64:73, :], offs[0:9, :],
                                            1.0, 0.0, MUL,
                                            mybir.AluOpType.max)
                    nc.vector.tensor_scalar(gsb[0:9, :], offc[:],
                                            -1.0, 0.0, MUL,
                                            mybir.AluOpType.max)
                    nc.scalar.activation(tmpb[0:9, :], offc[:], ABS)
                    nc.vector.tensor_scalar(gsb[32:41, :], tmpb[0:9, :],
                                            -1.0, 1.0, MUL, ADD)
                    nc.vector.tensor_scalar(gsb[64:73, :], offc[:],
                                            1.0, 0.0, MUL,
                                            mybir.AluOpType.max)

                    # skew-store F: f_{dr}(off_r[n])(u) at
                    #   fdram[3n+i_dr, MARG + 66*i_nr + i_nc + 66*i_dr + u]
                    # merged: one DMA per i_dr over (i_nr, i_nc, u)
                    for i_dr in range(3):
                        dst = _mk_src(
                            fdram, [[9 * FW + 66, 3], [3 * FW + 1, 3], [1, U]],
                            i_dr * (FW + 66) + MARG)
                        nc.sync.dma_start(out=dst,
                                          in_=fsb[32 * i_dr: 32 * i_dr + 9, :])
                    # skew-store G: g_{dc}(off_c[n])(u) at
                    #   gdram[4n+i_dc, MARG + 66*i_nr + i_nc + i_dc + u]
                    # merged: one DMA per jc over (i_nr, i_nc, u)
                    for jc, grow in [(1, 64), (2, 32), (3, 0)]:
                        dst = _mk_src(
                            gdram, [[12 * FW + 66, 3], [4 * FW + 1, 3], [1, U]],
                            jc * FW + MARG + (3 - jc))
                        nc.sync.dma_start(out=dst, in_=gsb[grow: grow + 9, :])
                    # shifted reads -> P_r, P_c in slot order l = 12n + 4*i_dr + i_dc
                    # (prt/pct live in prtp, which the b5 pools never reuse,
                    # so b5 allocation isn't gated on the sw multiply)
                    prt = prtp.tile([108, VPW], BF, name="prt")
                    pct = prtp.tile([108, VPW], BF, name="pct")
                    pct_eng = [nc.gpsimd, nc.gpsimd, nc.sync]
                    for n in range(TAPS):
                        src = _mk_src(fdram, [[FW, 3], [1, 4], [1, VPW]],
                                      3 * n * FW + MARG - 1)
                        nc.sync.dma_start(out=prt[12 * n:12 * n + 12, :],
                                          in_=src)
                        for i_dr in range(3):
                            src = _mk_src(gdram, [[FW, 4], [1, VPW]],
                                          4 * n * FW + MARG + 2 - 66 * i_dr)
                            pct_eng[i_dr].dma_start(
                                out=pct[12 * n + 4 * i_dr:
                                        12 * n + 4 * i_dr + 4, :],
                                in_=src)
                    # sw multiply on gpsimd: its queue is otherwise idle, so
                    # it fires at real semaphore time instead of queuing
                    # behind vector's cast stream.  (Engine-op sources must
                    # start at partition 0, so no per-tap chunking.)
                    nc.gpsimd.tensor_tensor(sw[0:108, :], prt[:], pct[:], MUL)

                # ---- y channel mixes (PE filler; overlaps staging) ----
                # five 1-bank 512-col psum chunks (2 pre-opened + 3 post-
                # opsum): cb-outer emits runs of 5 same-lhsT matmuls (amortized
                # LDWEIGHTS) and each chunk's slot frees after one fast 512-col
                # cast, so the psy pipeline never stalls on cast completion.
                YCH = [(0, 512), (512, 1024), (1024, 1536), (1536, 2048),
                       (2048, 2304)]
                for vt in range(VT):
                    v0 = vt * P
                    y = ypool.tile([P, TAPS * 256], BF, tag=f"y{vt}",
                                   name=f"y{vt}")
                    ps = [ypsA.tile([P, 512], F32, tag="psyA",
                                    name=f"psy{vt}_0"),
                          ypsA.tile([P, 512], F32, tag="psyA",
                                    name=f"psy{vt}_1"),
                          ypsB.tile([P, 512], F32, tag="psyB",
                                    name=f"psy{vt}_2"),
                          ypsB.tile([P, 512], F32, tag="psyB",
                                    name=f"psy{vt}_3"),
                          ypsB.tile([P, 512], F32, tag="psyB",
                                    name=f"psy{vt}_4")]
                    for cb in range(CB):
                        for ci, (a, b) in enumerate(YCH):
                            nc.tensor.matmul(ps[ci][:, :b - a],
                                             res2b[cb][:, v0:v0 + P],
                                             dwt[cb][:, a:b],
                                             start=(cb == 0),
                                             stop=(cb == CB - 1))
                    for ci, (a, b) in enumerate(YCH):
                        if (vt + ci) % 2 == 0:
                            nc.vector.tensor_copy(y[:, a:b], ps[ci][:, :b - a])
                        else:
                            nc.scalar.activation(y[:, a:b], ps[ci][:, :b - a],
                                                 IDENT)
                    ys.append(y)
                ypsB_cm.__exit__(None, None, None)
                ypsA_cm.__exit__(None, None, None)

            # ============ phase 5: B5 build + banded matmuls ============
            with tc.tile_pool(name="b5apool", bufs=11) as b5apool, \
                 tc.tile_pool(name="b5bpool", bufs=11) as b5bpool, \
                 tc.tile_pool(name="swtpool", bufs=4) as swtpool:

                # B5 build per vtile (transpose -> swT -> scatter); high
                # priority so the serial gpsimd chain starts asap.
                built = {}
                with tc.high_priority():
                    for vt in range(VT):
                        v0 = vt * P
                        pst = tpsum.tile([P, P], BF, tag="pst")
                        nc.tensor.transpose(pst[:, :108], sw[0:108, v0:v0 + P],
                                            ident[:108, :108])
                        swT = swtpool.tile([P, 108], BF, tag="swT")
                        if vt % 2 == 0:
                            nc.vector.tensor_copy(swT[:], pst[:, :108])
                        else:
                            nc.scalar.activation(swT[:], pst[:, :108], IDENT)
                        b5a = b5apool.tile([P, 5 * BW], BF, tag="b5a")
                        b5b = b5bpool.tile([P, 4 * BW], BF, tag="b5b")
                        nc.gpsimd.local_scatter(b5a[:], swT[:, 0:60],
                                                iotaA[:], channels=P,
                                                num_elems=5 * BW, num_idxs=60)
                        nc.gpsimd.local_scatter(b5b[:], swT[:, 60:108],
                                                iotaB[:], channels=P,
                                                num_elems=4 * BW, num_idxs=48)
                        built[vt] = (b5a, b5b)

                # banded matmuls + residual + output
                if True:
                    with tc.tile_pool(name="bpsum", bufs=3, space="PSUM") as bpsum:
                        for (ob, rb) in UBLOCKS:
                            U0, UW = ob * W, rb * W
                            mms = _ublock_mms(ob, rb)
                            for mb in range(CB):
                                psb = bpsum.tile([P, UW], F32, tag="psb")
                                nc.vector.memset(psb[:], 0)
                                for j, (vt, n, w0, lo, hi) in enumerate(mms):
                                    b5a, b5b = built[vt]
                                    lhsT = ys[vt][:, n * 256 + mb * P:
                                                  n * 256 + mb * P + P]
                                    for (sa, sb_) in _split_at_banks(lo, hi, U0):
                                        if n < 5:
                                            rhs = b5a[:, n * BW + sa - w0:
                                                      n * BW + sb_ - w0]
                                        else:
                                            rhs = b5b[:, (n - 5) * BW + sa - w0:
                                                      (n - 5) * BW + sb_ - w0]
                                        nc.tensor.matmul(
                                            psb[:, sa - U0:sb_ - U0],
                                            lhsT, rhs, start=False,
                                            stop=(j == len(mms) - 1),
                                            skip_group_check=True)
                                outt = outp.tile([P, 726], F32, tag="outstage")
                                nc.vector.tensor_add(
                                    outt[:, :UW], psb[:],
                                    xsb[mb][:, 1 + (ob + 4) * W:
                                            1 + (ob + 4) * W + UW])
                                if (ob, rb) == UBLOCKS[-1]:
                                    rsp = [(r, 2) for r in range(0, rb, 2)]
                                    engs = [nc.sync, nc.scalar, nc.gpsimd,
                                            nc.sync, nc.scalar]
                                else:
                                    rsp = [(0, 4), (4, 4), (8, rb - 8)]
                                    engs = [nc.sync, nc.sync, nc.sync]
                                for ei, (r0c, rc) in enumerate(rsp):
                                    engs[ei].dma_start(
                                        out=out_d.ap()[mb, :, ob + r0c:
                                                       ob + r0c + rc, :],
                                        in_=outt[:, r0c * W:(r0c + rc) * W]
                                        .rearrange("p (r w) -> p r w",
                                                   w=W)[:, :, 1:65])

    nc.finalize()
    return nc


def _pack_inputs(x, w1, b1, w2, b2, p_w, p_b, dw):
    """Build the 8 per-core input maps (numpy only)."""
    x = np.asarray(x, np.float32)

    def pack_w(w, mout):
        w = np.asarray(w, np.float32)
        out = np.empty((CB, TAPS, P, mout), bf16)
        for cb in range(CB):
            for t in range(TAPS):
                out[cb, t] = w[:, cb * P:(cb + 1) * P, t // 3, t % 3].T.astype(bf16)
        return out

    w1t = pack_w(w1, 256)
    w2t = pack_w(w2, 256)
    pwt = pack_w(p_w, 18)
    dwt = np.empty((CB, P, TAPS * 256), bf16)
    dwf = np.asarray(dw, np.float32)
    for cb in range(CB):
        for t in range(TAPS):
            dwt[cb, :, t * 256:(t + 1) * 256] = \
                dwf[:, cb * P:(cb + 1) * P, t // 3, t % 3].T.astype(bf16)
    b1p = np.ascontiguousarray(np.asarray(b1, np.float32).reshape(CB, P, 1))
    b2p = np.ascontiguousarray(np.asarray(b2, np.float32).reshape(CB, P, 1))
    pbp = np.ascontiguousarray(np.asarray(p_b, np.float32).reshape(18, 1))
    zerosp = np.zeros((36, FW), bf16)

    maps = []
    for core in range(N_CORES):
        b, half = core // 2, core % 2
        r0 = 32 * half
        slab = np.zeros((CB, P, TS, W), np.float32)
        g0, g1 = max(0, r0 - 4), min(64, r0 + 36)
        t0 = g0 - (r0 - 4)
        for cb in range(CB):
            slab[cb, :, t0:t0 + (g1 - g0), 1:65] = \
                x[b, cb * P:(cb + 1) * P, g0:g1, :]
        xsv = np.zeros((CB, P, XL), np.float32)
        xsv[:, :, 1:1 + TS * W] = slab.reshape(CB, P, TS * W)
        maskr = np.zeros((P, TS), bf16)
        valid = np.array([1.0 if 0 <= r0 - 4 + t < 64 else 0.0
                          for t in range(TS)], np.float32)
        maskr[:] = valid.astype(bf16)[None, :]
        maps.append({
            "xsb": xsv.astype(bf16), "maskr": maskr,
            "w1t": w1t, "w2t": w2t, "pwt": pwt, "dwt": dwt,
            "b1": b1p, "b2": b2p, "pb": pbp, "zeros": zerosp,
        })
    return maps


def get_program():
    if "nc" not in _CACHE:
        _CACHE["nc"] = _build_program()
    return _CACHE["nc"]


def _ensure_ntff_hook():
    """The image's antenv lacks axon_hooks; inject a shim and register the
    NTFF profiling hook so trace=True works under axon."""
    import sys, types
    import antenv
    if "antenv.axon_hooks" in sys.modules:
        return
    mod = types.ModuleType("antenv.axon_hooks")
    mod._hook = None
    def set_axon_ntff_profile_hook(h):
        mod._hook = h
    def get_axon_ntff_profile_hook():
        return mod._hook
    mod.set_axon_ntff_profile_hook = set_axon_ntff_profile_hook
    mod.get_axon_ntff_profile_hook = get_axon_ntff_profile_hook
    sys.modules["antenv.axon_hooks"] = mod
    antenv.axon_hooks = mod
    try:
        from trn_agent_boot.trn_boot import _ntff_profile_via_ctypes
        hook = _ntff_profile_via_ctypes("/opt/axon/libaxon_pjrt.so")
        if hook is not None:
            set_axon_ntff_profile_hook(hook)
    except Exception as e:
        print("ntff hook setup failed:", e)


def kernel(x, w1, b1, w2, b2, p_w, p_b, dw):
    global LAST_RESULTS
    nc = get_program()
    maps = _pack_inputs(x, w1, b1, w2, b2, p_w, p_b, dw)
    trace = os.environ.get("DEHAZE_TRACE") == "1"
    if trace:
        _ensure_ntff_hook()
    res = run_bass_kernel_spmd(nc, maps, core_ids=list(range(N_CORES)),
                               trace=trace)
    LAST_RESULTS = res
    out = np.empty((4, 256, 64, 64), np.float32)
    for core in range(N_CORES):
        b, half = core // 2, core % 2
        o = res.results[core]["out"]  # [CB, P, RO, 64]
        out[b, :, 32 * half:32 * half + 32, :] = o.reshape(256, 32, 64)
    return out
